# revision 6
# baseline (speedup 1.0000x reference)
"""GQA attention with KV cache, tensor-parallel over 8 TRN2 NeuronCores.

Problem shapes (hardcoded): H=32 q-heads, KVH=8 kv-heads, D=128 head_dim,
DIM=4096, T=256 new tokens, MAX_SEQ=8192, pos=4096 (runtime input).

Sharding: head-parallel. Core c owns q-heads 4c..4c+3 and kv-head c:
  wq rows  [c*512:(c+1)*512], wk/wv rows [c*128:(c+1)*128],
  wo cols  [c*512:(c+1)*512], k/v_cache head c.
Each core computes a full (T, DIM) partial of the output projection;
the host sums the 8 partials (the TP all-reduce) and reshapes.

v3 structure (driven by v1/v2 traces):
- The exp/ACT stream is the bound of the attention loop (~39us of ACT at
  1 elem/lane/cycle); everything else is arranged to start it early and
  keep it saturated.
- Each dma_start occupies its issuing engine ~0.6us (HWDGE descriptor
  generation), so transfers are few and large, split across BOTH HWDGE
  rings (sync+scalar) which round-robin at packet granularity:
    sync:   wq k-chunks -> kcT halves -> wkv halves -> wo quarters
    scalar: xT k-chunks -> q-tables -> vc halves -> k-tables
  FIFO order per ring is the priority mechanism; the q-projection inputs
  (xT+wq, 6.3MB) own the full wire until they land (~20us).
- phase A: q = x @ wq.T only (PSUM over 32 k-tiles), full-width RoPE
  (6 DVE ops), PE-transpose into qrT [d, h*T+t].
- phase B s-loop (34 key blocks): scoresT = kT.T @ qrT (2 N=512 MMs),
  probsT = exp(scoresT-4) (one 1024-wide ACT op from PSUM),
  acc_sum += probsT (DVE), pv += v.T @ probsT (2 N=512 MMs, PSUM-
  accumulated). The k|v projection (64 N=256 MMs into one rotating PSUM
  bank), its evictions, k-RoPE and cache-tail writes are woven into the
  loop's PE slack at ~3 MMs per s-slot.
- softmax denominators: ones-col matmul partition-reduce, fast approx
  reciprocal (fp16), then a K=1 ones-row matmul BROADCASTS rinv across
  partitions (the gpsimd partition_broadcast costs ~3us/op; the PE does
  it in ~0.2us), scalar-engine eviction, DVE scale of pv.
- phase C: out = sum_h attnT_h.T @ wo_h, n-major; each 512-col PSUM
  chunk is evicted right after its 4 h-MMs (DVE/ACT alternating) and
  DMA'd per 1024-col pair on alternating rings.

All matmul operands fp16 (f32 PSUM accumulation); rel err ~2e-3 vs the
2e-2 budget. Rotary tables fp16, cos|sin merged per tensor (scale and
the rotate-half sign are host-folded).
"""

import numpy as np

import concourse.mybir as mybir
import concourse.tile as tile
from concourse import bacc
from concourse.bass_utils import run_bass_kernel_spmd
from concourse.masks import make_identity

H, KVH, D = 32, 8, 128
DIM, T, MAX_SEQ = 4096, 256, 8192
NC_ = 8                      # cores
HL = H // NC_                # local q heads = 4
SCALE = 1.0 / float(np.sqrt(D))
EXP_BIAS = -4.0              # cancels in softmax normalization

F32 = mybir.dt.float32
FP16 = mybir.dt.float16
NP_FP16 = np.float16

_BUILD_CACHE: dict = {}


def _build(pos: int):
    """Trace + compile the per-core program. Same program runs on all 8
    cores (SPMD); only the DRAM input contents differ."""
    S_OLD = pos              # cached tokens
    S = pos + T              # total keys
    NB_OLD = S_OLD // 128    # cached s-blocks (32)
    NB = S // 128            # total s-blocks (34)
    NKT = DIM // 128         # contraction k-tiles (32)
    NT = T // 128            # t-tiles (2)

    nc = bacc.Bacc("TRN2", target_bir_lowering=False, debug=False)

    d_xT = nc.dram_tensor("xTp", (128, NKT * T), FP16, kind="ExternalInput")
    d_wq = nc.dram_tensor("wqp", (128, NKT * HL * D), FP16,
                          kind="ExternalInput")
    d_wkv = nc.dram_tensor("wkvp", (128, NKT * 2 * D), FP16,
                           kind="ExternalInput")
    d_wo = nc.dram_tensor("wop", (128, 8 * HL * 512), FP16,
                          kind="ExternalInput")
    d_kcT = nc.dram_tensor("kcT", (D, S_OLD), FP16, kind="ExternalInput")
    d_vc = nc.dram_tensor("vcp", (128, NB_OLD * D), FP16,
                          kind="ExternalInput")
    d_csq = nc.dram_tensor("csq", (T, 2 * HL * D), FP16,
                           kind="ExternalInput")
    d_csk = nc.dram_tensor("csk", (T, 2 * D), FP16, kind="ExternalInput")
    d_out = nc.dram_tensor("out", (T, DIM), FP16, kind="ExternalOutput")

    with tile.TileContext(nc) as tc:
        with (
            tc.tile_pool(name="persist", bufs=1) as pp,
            tc.tile_pool(name="wstream", bufs=1) as wp,
            tc.tile_pool(name="small", bufs=3) as sp,
            tc.tile_pool(name="probs", bufs=4) as prp,
            tc.tile_pool(name="wotile", bufs=4) as wop,
        ):
            # persistent activations / streams
            xT = pp.tile([128, NKT, T], FP16, tag="xT")
            wq_sb = wp.tile([128, NKT, HL * D], FP16, tag="wq")
            wkv_sb = wp.tile([128, NKT, 2 * D], FP16, tag="wkv")
            kT_all = pp.tile([128, S], FP16, tag="kT")           # [d, s]
            v_all = pp.tile([128, NB * D], FP16, tag="vall")
            qrT = pp.tile([128, HL * T], FP16, tag="qrT")        # [d, h*T+t]
            acc_sum = pp.tile([128, HL * T], FP16, tag="accsum")
            attnT = pp.tile([128, HL * T], FP16, tag="attnT")
            rinv = pp.tile([1, HL * T], F32, tag="rinv")
            rinv_bc = pp.tile([128, HL * T], F32, tag="rinvbc")
            csq = [pp.tile([128, 2 * HL * D], FP16, tag=f"csq{i}",
                           name=f"cs_q{i}") for i in range(NT)]
            csk = [pp.tile([128, 2 * D], FP16, tag=f"csk{i}",
                           name=f"cs_k{i}") for i in range(NT)]
            kv_sb = [pp.tile([128, 2 * D], FP16, tag=f"kvsb{i}",
                             name=f"kv_sb{i}") for i in range(NT)]

            # ---- DMA issue: both HWDGE rings, FIFO order = priority ----
            # sync ring: wq chunks -> kcT halves -> wkv halves -> wo
            WQC = [(0, 4), (4, 12), (12, 20), (20, 28), (28, 32)]
            for k0, k1 in WQC:
                nc.sync.dma_start(
                    wq_sb[:, k0:k1, :],
                    d_wq.ap()[:, k0 * HL * D:k1 * HL * D]
                    .rearrange("p (c w) -> p c w", w=HL * D))
            # scalar ring: xT chunks -> q tables -> vc halves -> k tables
            XTC = [(0, 8), (8, 20), (20, 32)]
            for k0, k1 in XTC:
                nc.scalar.dma_start(
                    xT[:, k0:k1, :],
                    d_xT.ap()[:, k0 * T:k1 * T]
                    .rearrange("p (k t) -> p k t", t=T))
            for i in range(NT):
                nc.scalar.dma_start(csq[i][:],
                                    d_csq.ap()[i * 128:(i + 1) * 128, :])
            KCC = S_OLD // 2
            for c in range(2):
                nc.sync.dma_start(kT_all[:, c * KCC:(c + 1) * KCC],
                                  d_kcT.ap()[:, c * KCC:(c + 1) * KCC])
            VCC = NB_OLD // 2 * D
            for c in range(2):
                nc.scalar.dma_start(v_all[:, c * VCC:(c + 1) * VCC],
                                    d_vc.ap()[:, c * VCC:(c + 1) * VCC])
            for c in range(2):
                k0, k1 = c * (NKT // 2), (c + 1) * (NKT // 2)
                nc.sync.dma_start(
                    wkv_sb[:, k0:k1, :],
                    d_wkv.ap()[:, k0 * 2 * D:k1 * 2 * D]
                    .rearrange("p (c w) -> p c w", w=2 * D))
            for i in range(NT):
                nc.scalar.dma_start(csk[i][:],
                                    d_csk.ap()[i * 128:(i + 1) * 128, :])
            wo_ch = []
            for n in range(4):
                w_sb = wop.tile([128, 2, HL, 512], FP16, tag="wot",
                                name=f"wo_n{n}")
                nc.sync.dma_start(
                    w_sb[:],
                    d_wo.ap()[:, n * 2 * HL * 512:(n + 1) * 2 * HL * 512]
                    .rearrange("p (c h m) -> p c h m", h=HL, m=512))
                wo_ch.append(w_sb)

            # ---- constants (other engines; after DMA issue) ----
            ident = pp.tile([128, 128], FP16, tag="ident")
            scr_i = sp.tile([128, 128], F32, tag="cscr", name="scr_ident")
            make_identity(nc, scr_i[:])
            nc.vector.tensor_copy(ident[:], scr_i[:])
            ones_col = pp.tile([128, 1], FP16, tag="ones")
            ones_row = pp.tile([1, 128], F32, tag="onesr")
            scr_o = sp.tile([128, 1], F32, tag="cscr1", name="scr_ones")
            nc.gpsimd.memset(scr_o[:], 1.0)
            nc.vector.tensor_copy(ones_col[:], scr_o[:])
            nc.gpsimd.memset(ones_row[:], 1.0)
            ebias = pp.tile([128, 1], F32, tag="ebias")
            nc.gpsimd.memset(ebias[:], EXP_BIAS)

            # ================= phase A: q projection + RoPE ===============
            with (
                tc.tile_pool(name="ps_proj", bufs=1, space="PSUM") as ps_pj,
                tc.tile_pool(name="ps_tr", bufs=2, space="PSUM") as ps_tr,
            ):
                for w_ in range(20):
                    pw = ps_tr.tile([128, 128], F32, tag="wa",
                                    name=f"warm{w_}")
                    nc.tensor.matmul(pw[:], ident[:], ident[:],
                                     start=True, stop=True)
                ps_q = [ps_pj.tile([128, HL * D], F32, tag=f"psq{i}",
                                   name=f"ps_q{i}") for i in range(NT)]
                for k in range(NKT):
                    for i in range(NT):
                        nc.tensor.matmul(
                            ps_q[i][:], xT[:, k, i * 128:(i + 1) * 128],
                            wq_sb[:, k, :],
                            start=(k == 0), stop=(k == NKT - 1))
                # full-width RoPE per t-tile; transposes eagerly after
                W = HL * D

                def swapv(ap, nh):
                    v = ap.rearrange("p (h a b) -> p h a b", a=2, b=64)
                    return v[:, :, ::-1, :]

                for i in range(NT):
                    m1 = sp.tile([128, W], FP16, tag="m1")
                    nc.vector.tensor_mul(m1[:], ps_q[i][:], csq[i][:, 0:W])
                    m2 = sp.tile([128, W], FP16, tag="m2")
                    nc.vector.tensor_mul(m2[:], swapv(ps_q[i][:], HL),
                                         csq[i][:, W:2 * W])
                    qr_nat = sp.tile([128, W], FP16, tag="qrnat")
                    nc.vector.tensor_add(qr_nat[:], m1[:], m2[:])
                    for h in range(HL):
                        p = ps_tr.tile([128, 128], FP16, tag="tr")
                        nc.tensor.transpose(
                            p[:], qr_nat[:, h * 128:(h + 1) * 128], ident[:])
                        nc.vector.tensor_copy(
                            qrT[:, h * T + i * 128: h * T + (i + 1) * 128],
                            p[:])

            # ================= phase B: attention =========================
            def rope_swap_sb(ap):
                v = ap.rearrange("p (a b) -> p a b", a=2, b=64)
                return v[:, ::-1, :]

            # weave schedule: slot s -> ("kv", grp, [ks]) | ("evict", grp)
            # | ("rope", i)
            weave: dict = {}
            for grp in range(2):
                base = 6 + grp * 12
                ks = list(range(NKT))
                for j in range(11):
                    weave.setdefault(base + j, []).append(
                        ("kv", grp, ks[3 * j:3 * j + 3]))
                weave.setdefault(base + 11, []).append(("evict", grp))
            weave.setdefault(30, []).append(("rope", 0))
            weave.setdefault(31, []).append(("rope", 1))

            with tc.tile_pool(name="ps_pv", bufs=1, space="PSUM") as ps_pv:
                pv = ps_pv.tile([128, HL * T], F32, tag="pv")
                with (
                    tc.tile_pool(name="ps_sc", bufs=2, space="PSUM") as ps_sc,
                    tc.tile_pool(name="ps_sm", bufs=1, space="PSUM") as ps_sm,
                ):
                    ps_kv = [None, None]

                    def do_weave(s):
                        for item in weave.get(s, ()):
                            if item[0] == "kv":
                                _, grp, ks = item
                                if not ks:
                                    continue
                                if ps_kv[grp] is None:
                                    ps_kv[grp] = ps_sm.tile(
                                        [128, 2 * D], F32, tag="kv",
                                        name=f"ps_kv{grp}")
                                for k in ks:
                                    nc.tensor.matmul(
                                        ps_kv[grp][:],
                                        xT[:, k, grp * 128:(grp + 1) * 128],
                                        wkv_sb[:, k, :],
                                        start=(k == 0), stop=(k == NKT - 1))
                            elif item[0] == "evict":
                                grp = item[1]
                                nc.vector.tensor_copy(kv_sb[grp][:],
                                                      ps_kv[grp][:])
                            else:
                                i = item[1]
                                km1 = sp.tile([128, D], FP16, tag="km1")
                                nc.vector.tensor_mul(
                                    km1[:], kv_sb[i][:, 0:D],
                                    csk[i][:, 0:D])
                                km2 = sp.tile([128, D], FP16, tag="km2")
                                nc.vector.tensor_mul(
                                    km2[:], rope_swap_sb(kv_sb[i][:, 0:D]),
                                    csk[i][:, D:2 * D])
                                kr_nat = sp.tile([128, D], FP16, tag="krnat")
                                nc.vector.tensor_add(kr_nat[:], km1[:], km2[:])
                                p = ps_sm.tile([128, 128], FP16, tag="ktr",
                                               name=f"ktr{i}")
                                nc.tensor.transpose(p[:], kr_nat[:], ident[:])
                                nc.vector.tensor_copy(
                                    kT_all[:, S_OLD + i * 128:
                                           S_OLD + (i + 1) * 128], p[:])
                                nc.vector.tensor_copy(
                                    v_all[:, (NB_OLD + i) * D:
                                          (NB_OLD + i + 1) * D],
                                    kv_sb[i][:, D:2 * D])

                    for s in range(NB):
                        do_weave(s)
                        sc = ps_sc.tile([128, HL * T], F32, tag="sc")
                        for half in range(2):
                            nc.tensor.matmul(
                                sc[:, half * 512:(half + 1) * 512],
                                kT_all[:, s * 128:(s + 1) * 128],
                                qrT[:, half * 512:(half + 1) * 512],
                                start=True, stop=True)
                        pb = prp.tile([128, HL * T], FP16, tag="pb")
                        nc.scalar.activation(
                            pb[:], sc[:], mybir.ActivationFunctionType.Exp,
                            bias=ebias[:])
                        if s == 0:
                            nc.vector.tensor_copy(acc_sum[:], pb[:])
                        else:
                            nc.vector.tensor_add(acc_sum[:], acc_sum[:], pb[:])
                        for half in range(2):
                            hs = slice(half * 512, (half + 1) * 512)
                            nc.tensor.matmul(
                                pv[:, hs],
                                v_all[:, s * D:(s + 1) * D],
                                pb[:, hs],
                                start=(s == 0), stop=(s == NB - 1))

                # denominators: colsum (ones-col MM), approx reciprocal
                # (fp16), K=1 ones-row MM broadcast, scalar eviction, scale.
                with (
                    tc.tile_pool(name="ps_post", bufs=1, space="PSUM") as psp,
                    tc.tile_pool(name="ps_warm", bufs=2, space="PSUM") as pw_p,
                ):
                    sm = psp.tile([1, HL * T], F32, tag="sm")
                    rbc_ps = [None, None]
                    for half in range(2):
                        hs = slice(half * 512, (half + 1) * 512)
                        nc.tensor.matmul(
                            sm[:, hs], ones_col[:], acc_sum[:, hs],
                            start=True, stop=True)
                        nc.vector.reciprocal_approx_fast(
                            rinv[:, hs], sm[:, hs])
                        rbc_ps[half] = pw_p.tile([128, 512], F32, tag="rbc",
                                                 name=f"rbc{half}")
                        nc.tensor.matmul(rbc_ps[half][:], ones_row[:],
                                         rinv[:, hs], start=True, stop=True)
                        nc.scalar.copy(rinv_bc[:, hs], rbc_ps[half][:])
                    # keep the PE busy (HAM warm) across the serial tail
                    for w_ in range(5):
                        pw = pw_p.tile([128, 512], F32, tag="wk",
                                       name=f"wk{w_}")
                        nc.tensor.matmul(pw[:], ident[:], qrT[:, 0:512],
                                         start=True, stop=True)
                    for half in range(2):
                        hs = slice(half * 512, (half + 1) * 512)
                        nc.vector.tensor_mul(
                            attnT[:, hs], pv[:, hs], rinv_bc[:, hs])

            # ================= phase C: output projection =================
            # n-major: each po chunk completes after its 4 h-matmuls, is
            # evicted (DVE/ACT alternating); DMA per 1024-col pair.
            with (
                tc.tile_pool(name="ps_wo", bufs=8, space="PSUM") as ps_wo,
                tc.tile_pool(name="obp", bufs=4) as obp,
            ):
                for nh in range(2):
                    for i in range(NT):
                        for np_ in range(2):
                            ob = obp.tile([128, 1024], FP16, tag="ob",
                                          name=f"ob_{nh}_{i}_{np_}")
                            for half in range(2):
                                n_ = np_ * 2 + half
                                n = nh * 4 + n_
                                po = ps_wo.tile([128, 512], F32, tag="po",
                                                name=f"po_{nh}_{i}_{n_}")
                                for h in range(HL):
                                    nc.tensor.matmul(
                                        po[:],
                                        attnT[:, h * T + i * 128:
                                              h * T + (i + 1) * 128],
                                        wo_ch[n // 2][:, n % 2, h, :],
                                        start=(h == 0), stop=(h == HL - 1))
                                if half == 0:
                                    nc.vector.tensor_copy(
                                        ob[:, 0:512], po[:])
                                else:
                                    nc.scalar.copy(ob[:, 512:1024], po[:])
                            ring = nc.scalar if (i + np_) % 2 else nc.sync
                            ring.dma_start(
                                d_out.ap()[i * 128:(i + 1) * 128,
                                           nh * 2048 + np_ * 1024:
                                           nh * 2048 + (np_ + 1) * 1024],
                                ob[:])

    nc.compile()
    return nc


def _pmajor(a, np_, inner):
    """(np_*128, inner) row-blocked array -> (128, np_*inner) partition-
    major: out[p, j*inner:(j+1)*inner] = a[j*128 + p, :]."""
    return np.ascontiguousarray(
        a.reshape(np_, 128, inner).transpose(1, 0, 2).reshape(
            128, np_ * inner))


def _prep_inputs(x, cos, sin, wq, wk, wv, wo, k_cache, v_cache, pos):
    """Host-side shard + layout/dtype prep (no arithmetic beyond scaling
    the rotary tables). Returns in_maps for the 8 cores."""
    f = np.float32
    pos = int(pos)
    x2d = np.asarray(x, dtype=f).reshape(T, DIM)
    xTp = _pmajor(np.ascontiguousarray(x2d.T).astype(NP_FP16), DIM // 128, T)
    cos = np.asarray(cos, dtype=np.float64)
    sin = np.asarray(sin, dtype=np.float64)
    sgn = np.concatenate([-np.ones(D // 2), np.ones(D // 2)])
    csq = np.ascontiguousarray(np.concatenate(
        [np.tile(cos * SCALE, (1, HL)), np.tile(sin * sgn * SCALE, (1, HL))],
        axis=1), dtype=NP_FP16)                              # (T, 2*HL*D)
    csk = np.ascontiguousarray(np.concatenate(
        [cos, sin * sgn], axis=1), dtype=NP_FP16)            # (T, 2*D)
    wq = np.asarray(wq, dtype=f)
    wk = np.asarray(wk, dtype=f)
    wv = np.asarray(wv, dtype=f)
    wo = np.asarray(wo, dtype=f)
    k_cache = np.asarray(k_cache, dtype=f)
    v_cache = np.asarray(v_cache, dtype=f)
    in_maps = []
    for c in range(NC_):
        wqT = wq[c * HL * D:(c + 1) * HL * D, :].T          # (DIM, 512)
        wkvT = np.concatenate([
            wk[c * D:(c + 1) * D, :].T,
            wv[c * D:(c + 1) * D, :].T], axis=1)            # (DIM, 256)
        woT = wo[:, c * HL * D:(c + 1) * HL * D].T          # (HL*D, DIM)
        # (128, 8*HL*512): partition d-within-head, then [n, h, m]
        wop_ = woT.astype(NP_FP16).reshape(HL, 128, 8, 512).transpose(
            1, 2, 0, 3).reshape(128, 8 * HL * 512)
        in_maps.append({
            "xTp": xTp,
            "wqp": _pmajor(wqT.astype(NP_FP16), DIM // 128, HL * D),
            "wkvp": _pmajor(wkvT.astype(NP_FP16), DIM // 128, 2 * D),
            "wop": np.ascontiguousarray(wop_),
            "kcT": np.ascontiguousarray(
                k_cache[c, :pos, :].T.astype(NP_FP16)),
            "vcp": _pmajor(v_cache[c, :pos, :].astype(NP_FP16),
                           pos // 128, D),
            "csq": csq, "csk": csk,
        })
    return in_maps


def run(trace=False, **inputs):
    """Build (cached), run on 8 cores, reduce. Returns (out, results)."""
    pos = int(inputs["pos"])
    if pos not in _BUILD_CACHE:
        _BUILD_CACHE[pos] = _build(pos)
    nc = _BUILD_CACHE[pos]
    in_maps = _prep_inputs(**inputs)
    res = run_bass_kernel_spmd(
        nc, in_maps, core_ids=list(range(NC_)), trace=trace)
    part = np.stack([np.asarray(r["out"], np.float32)
                     for r in res.results])               # (8, T, DIM)
    out = part.sum(axis=0, dtype=np.float32).reshape(1, T, DIM)
    return out, res


def kernel(**inputs):
    out, _ = run(trace=False, **inputs)
    return out
